# revision 34
# baseline (speedup 1.0000x reference)
"""GCN (2x GraphConv + BatchNorm + Linear) forward on 8 Trainium2 NeuronCores.

Sharding: data-parallel over the batch axis -- each core owns one whole graph,
so the gather/segment-sum stays core-local.  The big lin_W contraction is
reformulated per-channel:

  out[b,c] = sum_f a[f] * P[b,c,f] + sum_f d[f] * S[c,f] + lin_b[c]

where P[b,c,f] = sum_n h2[b,n,f] * lin_W[c, n*F+f], S[c,f] = sum_n lin_W[c,n*F+f],
and (a, d) are the BatchNorm affine coefficients derived from global mean/var.

v2 design (vs v1, 1368766 -> 745658 ns TimelineSim/core): fp16 path end to end.
  * Host relabels nodes per graph (balanced bin-packing on in-degree, exact
    repair) so each 128-node dst slice has exactly E/NS incident edges ->
    zero gather padding (EPS 2304 -> 2048).
  * xs/h1 stored in HBM as fp16 rows padded to 256B; dma_gather with
    elem_size=128 fp16.  Edge gathers are the hard floor: one 256B
    descriptor per edge (22.76ns/desc across 16 DMA engines) ~ 373us/layer.
  * One-hot built per 128-edge block via tensor_scalar is_equal on fp16
    (DVE 4x_2p mode, ~94ns/block vs ~2.2us/slice for the fp32 big build).
  * Scatter matmuls fp16 (cost keys off the MOVING operand dtype: 1 cyc/row
    vs 4 for fp32); conv via aggT-stationary avoids all transposes.
  * rs_in folded into the post-conv epilogue (per-node = per-partition there,
    single scalar_tensor_tensor); rs_out folded into xs / relu scale.
  * lin_W streamed as fp16 in node-major [N, C*F] layout (1280B descriptors,
    no sub-512B DMA penalty, 233 -> 58us); P / S / BN sums computed by three
    PSUM-accumulated matmuls per slice against st = [h2_s | ones] (Gram
    trick), interleaved with layer 2; per-slice h2 tiles (not one shared
    tensor) to avoid false WAR serialization; P diag extracted via a mask
    multiply + ones-matmul column sum at the end.
  * Small consts packed into two blob tensors (one DMA each).
  * Adjacent-src edge pairing: per slice, exactly npb*128 edge pairs whose
    srcs are consecutive node ids ship through npb blocks of 512B two-row
    descriptors (same per-descriptor price as 256B in the DMA model), the
    remaining edges through normal 256B descriptors.  Within each dst bin,
    nodes are ordered by a greedy max-co-occurrence chain (shared out-edge
    dst-bin sets, one 128x128 Gram matmul per bin) which lifts the per-slice
    pair supply from ~160 min to ~414 min -> npb=3 guaranteed: 2048 -> 1664
    descriptors/slice (-136us vs unpaired).  Automatic fallback npb 3->2->1->0
    if any slice of any graph lacks npb*128 pairs.
  * Layer loops software-pipelined (emission order): scatters of slice s,
    then aggTs copy + conv of s-1, then epilogue + lin_W tail of s-2, with
    lin_W prefetched at slice s -- PE/DVE never stall on each other's
    same-slice results, h1 still stored per group.
Remaining idle on the shared-DMA critical resource is ~31us: prep fill,
inter-layer barrier drain (DRAM RAW deps are NOT tracked by tile, so the
barriers are required), and final-group pipeline drain.
"""

import os
from contextlib import ExitStack

import numpy as np

import concourse.bass as bass
import concourse.tile as tile
from concourse import bacc, mybir
from concourse.bass_utils import run_bass_kernel_spmd

F32 = mybir.dt.float32
F16 = mybir.dt.float16
I16 = mybir.dt.int16
AF = mybir.ActivationFunctionType
ALU = mybir.AluOpType

BN_EPS = 1e-5

USE_BARRIERS = True
INTERLEAVE_TAIL = True


# ---------------------------------------------------------------- host prep

def _balanced_relabel(deg_in, n_nodes, nslice, src=None, dst=None):
    """Permutation old->new s.t. each of `nslice` bins of 128 consecutive new
    ids has (near-)equal total in-degree.  Greedy LPT + repair swaps.
    If (src, dst) given, each bin's members are ordered by a greedy
    max-co-occurrence chain (shared out-edge dst bins) so that consecutive
    ids are often co-sources of the same dst slice -> more 512B pair descs.

    Returns (perm, inv, max_bin_sum): new_id = perm[old_id], old = inv[new]."""
    cap = n_nodes // nslice
    target = int(deg_in.sum()) // nslice
    order = np.argsort(-deg_in, kind="stable")
    bin_sum = np.zeros(nslice, np.int64)
    bin_cnt = np.zeros(nslice, np.int64)
    bin_members = [[] for _ in range(nslice)]
    import heapq
    heap = [(0, 0, b) for b in range(nslice)]
    heapq.heapify(heap)
    for u in order:
        while True:
            s, c, b = heapq.heappop(heap)
            if bin_cnt[b] < cap and s == bin_sum[b]:
                break
        bin_members[b].append(u)
        bin_sum[b] += deg_in[u]
        bin_cnt[b] += 1
        if bin_cnt[b] < cap:
            heapq.heappush(heap, (int(bin_sum[b]), int(bin_cnt[b]), b))
    # repair: move toward exact balance with pair swaps between bins
    for _ in range(200):
        hi = int(np.argmax(bin_sum))
        lo = int(np.argmin(bin_sum))
        if bin_sum[hi] == target and bin_sum[lo] == target:
            break
        need = int(bin_sum[hi]) - target
        best = None
        lo_by_deg = {}
        for v in bin_members[lo]:
            lo_by_deg.setdefault(int(deg_in[v]), v)
        for u in bin_members[hi]:
            du = int(deg_in[u])
            for d in range(min(need, du - 1), 0, -1):
                v = lo_by_deg.get(du - d)
                if v is not None:
                    best = (u, v, d)
                    break
            if best:
                break
        if not best:
            break
        u, v, d = best
        bin_members[hi].remove(u)
        bin_members[lo].remove(v)
        bin_members[hi].append(v)
        bin_members[lo].append(u)
        bin_sum[hi] -= d
        bin_sum[lo] += d
    if src is not None:
        # bin_of for dst nodes (membership decided above)
        bin_of = np.empty(n_nodes, np.int64)
        for b in range(nslice):
            bin_of[bin_members[b]] = b
        hits = np.zeros((n_nodes, nslice), np.float32)
        hits[src, bin_of[dst]] = 1.0
        cap2 = n_nodes // nslice
        for b in range(nslice):
            nodes = np.asarray(bin_members[b])
            M = hits[nodes]                     # [cap, nslice]
            co = M @ M.T                        # shared-slice counts
            np.fill_diagonal(co, -1.0)
            used = np.zeros(len(nodes), bool)
            cur = 0
            order_l = [0]
            used[0] = True
            for _ in range(len(nodes) - 1):
                row = co[cur].copy()
                row[used] = -1.0
                cur = int(np.argmax(row))
                used[cur] = True
                order_l.append(cur)
            bin_members[b] = [int(nodes[i]) for i in order_l]
    perm = np.empty(n_nodes, np.int64)
    nxt = 0
    for b in range(nslice):
        for u in bin_members[b]:
            perm[u] = nxt
            nxt += 1
    inv = np.empty(n_nodes, np.int64)
    inv[perm] = np.arange(n_nodes)
    return perm, inv, int(bin_sum.max())


def _prep_graph(src, dst, n_nodes, eps=None):
    """Relabel + sort edges by dst slice, pad each slice to `eps` edges.

    Returns dict with idx16, dstloc, rs_out_t, rs_in_row, rs_out_col, perm,
    inv, max_cnt."""
    nslice = n_nodes // 128
    deg_out = np.bincount(src, minlength=n_nodes).astype(np.float32)
    deg_in = np.bincount(dst, minlength=n_nodes).astype(np.float32)
    rs_out = (1.0 / np.sqrt(np.maximum(deg_out, 1.0))).astype(np.float32)
    rs_in = (1.0 / np.sqrt(np.maximum(deg_in, 1.0))).astype(np.float32)

    perm, inv, max_cnt = _balanced_relabel(
        np.bincount(dst, minlength=n_nodes).astype(np.int64), n_nodes, nslice,
        src=src, dst=dst)
    src2 = perm[src]
    dst2 = perm[dst]
    rs_out2 = rs_out[inv]
    rs_in2 = rs_in[inv]

    out = {
        "perm": perm, "inv": inv, "max_cnt": max_cnt,
        "src2": src2, "dst2": dst2,
        "rs_out_col": rs_out2.reshape(nslice, 128).T.copy(),   # [128, NS]
        "rs_in_col": rs_in2.reshape(nslice, 128).T.copy(),     # [128, NS]
        "rs_or": rs_out2.reshape(128, -1).copy(),              # [128, N//128]
        "rs_out_flat": rs_out2,
    }
    if eps is None:
        return out
    _finish_prep(out, n_nodes, eps)
    return out


def _finish_prep(out, n_nodes, eps, npb=0):
    """Build idx16/dstloc.  npb>0: per slice exactly npb*128 adjacent-src
    pairs go through 512B two-row descriptors (npb pair blocks) and the rest
    through normal 256B descriptors.  Returns False if some slice lacks
    npb*128 pairs (caller falls back to a smaller npb)."""
    nslice = n_nodes // 128
    src2, dst2 = out["src2"], out["dst2"]
    sl = dst2 >> 7
    order = np.argsort(sl, kind="stable")
    counts = np.bincount(sl[order], minlength=nslice)
    assert counts.max() <= eps, (counts.max(), eps)
    starts = np.zeros(nslice + 1, np.int64)
    np.cumsum(counts, out=starts[1:])

    if npb == 0:
        src_s = src2[order]
        dst_s = dst2[order]
        sl_s = sl[order]
        npad = nslice * eps
        src_pad = np.zeros(npad, np.int16)
        dstloc_pad = np.full(npad, 128.0, np.float32)
        within = np.arange(len(src_s)) - starts[sl_s]
        pos = sl_s * eps + within
        src_pad[pos] = src_s.astype(np.int16)
        dstloc_pad[pos] = (dst_s & 127).astype(np.float32)
        out["idx16"] = np.tile(src_pad.reshape(-1, 16).T, (8, 1))
        out["dstloc"] = dstloc_pad.reshape(-1, 128).T.copy()
        return True

    NBLK = eps // 128
    NP = npb * 128
    NBU = NBLK - 2 * npb
    nu = NBU * 128
    idxU = np.zeros(nslice * nu, np.int16)
    idxP = np.zeros(nslice * NP, np.int16)
    dloc = np.full(nslice * eps, 128.0, np.float32)  # [s][blk 0..NBLK-1][lane]
    for s in range(nslice):
        eids = order[starts[s]:starts[s + 1]]
        srcs = src2[eids]
        so = np.argsort(srcs, kind="stable")
        ss = srcs[so]
        q = np.flatnonzero(ss[1:] - ss[:-1] == 1)
        keep = []
        last = -2
        for v in q:
            if v > last + 1:
                keep.append(v)
                last = v
                if len(keep) == NP:
                    break
        if len(keep) < NP:
            return False
        keep = np.asarray(keep)
        p1 = so[keep]
        p2 = so[keep + 1]
        e1 = eids[p1]
        e2 = eids[p2]
        m = np.zeros(len(eids), bool)
        m[p1] = True
        m[p2] = True
        rest = eids[~m]
        assert len(rest) <= nu, (len(rest), nu)
        idxU[s * nu:s * nu + len(rest)] = src2[rest].astype(np.int16)
        idxP[s * NP:(s + 1) * NP] = src2[e1].astype(np.int16)
        base = s * eps
        dloc[base:base + len(rest)] = (dst2[rest] & 127).astype(np.float32)
        d1 = (dst2[e1] & 127).astype(np.float32)
        d2 = (dst2[e2] & 127).astype(np.float32)
        for i in range(npb):
            o = base + nu + i * 256
            dloc[o:o + 128] = d1[i * 128:(i + 1) * 128]
            dloc[o + 128:o + 256] = d2[i * 128:(i + 1) * 128]
    allidx = np.concatenate([idxU, idxP])
    out["idx16"] = np.tile(allidx.reshape(-1, 16).T, (8, 1))
    out["dstloc"] = dloc.reshape(-1, 128).T.copy()
    return True


# ---------------------------------------------------------------- device build

def _build_program(n_nodes, feat, n_edges_pad_per_slice, n_cls, n_cores, gsl,
                   npb=0):
    NS = n_nodes // 128
    F = feat
    assert F == 64
    EPS = n_edges_pad_per_slice   # edges per slice, multiple of 128
    NBLK = EPS // 128
    NBU = NBLK - 2 * npb        # unpaired 256B-desc blocks per slice
    NPAD = NS * EPS
    # idx stream: NBU*128 unpaired + npb*128 pair descs per slice
    IDXN = NS * (NBU + npb) * 128
    CF = n_cls * F                # 640
    GSL = gsl
    assert NS % GSL == 0
    sizes = [GSL] * (NS // GSL)
    GROUPS = []
    acc = 0
    for sz in sizes:
        GROUPS.append((acc, sz))
        acc += sz

    nc = bacc.Bacc(
        "TRN2", target_bir_lowering=False, debug=False, num_devices=n_cores
    )

    NJ = n_nodes // 128
    # f32 const blob: b1b(F) | b2b(F) | rs_out(NS) | rs_in(NS) | rs_or(NJ) | dstloc(NPAD//128)
    BW32 = 2 * F + 2 * NS + NJ + NPAD // 128
    # f16 const blob: iota(128) | w1(F) | w2(F) | mask(CF) | ones(1)
    BW16 = 128 + 2 * F + CF + 1
    x_d = nc.dram_tensor("x", [n_nodes, F], F32, kind="ExternalInput")
    idx_d = nc.dram_tensor("idx", [128, IDXN // 16], I16, kind="ExternalInput")
    cb32_d = nc.dram_tensor("cb32", [128, BW32], F32, kind="ExternalInput")
    cb16_d = nc.dram_tensor("cb16", [128, BW16], F16, kind="ExternalInput")
    lw_d = nc.dram_tensor("lw16", [n_nodes, CF], F16, kind="ExternalInput")

    # out layout: P(CF) | S(CF) | s1(F) | s2(F)
    out_d = nc.dram_tensor("out", [1, 2 * CF + 2 * F], F32, kind="ExternalOutput")

    debug = bool(os.environ.get("GCN_DEBUG"))
    kind_i = "ExternalOutput" if debug else "Internal"
    xs_d = nc.dram_tensor("xs_i", [n_nodes, 128], F16, kind=kind_i)
    h1_d = nc.dram_tensor("h1_i", [n_nodes, 128], F16, kind=kind_i)
    h2_d = (nc.dram_tensor("h2_i", [128, NS * 65], F16, kind="ExternalOutput")
            if debug else None)

    with tile.TileContext(nc) as tc, ExitStack() as ctx:
        cpool = ctx.enter_context(tc.tile_pool(name="const", bufs=1))
        cb32 = cpool.tile([128, BW32], F32, tag="cb32")
        cb16 = cpool.tile([128, BW16], F16, tag="cb16")
        idx_sb = cpool.tile([128, IDXN // 16], I16, tag="idx")
        PB = NS * NBU * 8   # idx col base of the pair region
        for t, d in [(cb32, cb32_d), (cb16, cb16_d), (idx_sb, idx_d)]:
            nc.sync.dma_start(t[:], d.ap())
        o = 0
        b1_sb = cb32[:, o:o + F]; o += F
        b2_sb = cb32[:, o:o + F]; o += F
        rs_out_sb = cb32[:, o:o + NS]; o += NS
        rs_in_sb = cb32[:, o:o + NS]; o += NS
        rs_or_sb = cb32[:, o:o + NJ]; o += NJ
        dstloc_sb = cb32[:, o:o + NPAD // 128]; o += NPAD // 128
        o = 0
        iota_sb = cb16[:, o:o + 128]; o += 128
        w1_sb = cb16[0:F, o:o + F]; o += F
        w2_sb = cb16[0:F, o:o + F]; o += F
        mask_sb = cb16[0:F, o:o + CF]; o += CF
        ones_sb = cb16[0:F, o:o + 1]; o += 1

        # ---- prep: xs16 = (x * rs_out) as fp16, store to HBM 256B rows.
        # Contiguous load: partition q holds nodes [q*NJ, (q+1)*NJ) (x viewed
        # as [(q j) f]); rs_or[q, j] = rs_out[node q*NJ+j].
        PCH = 16 if NJ % 16 == 0 else (8 if NJ % 8 == 0 else 1)
        PW = NJ // PCH
        with tc.tile_pool(name="prep", bufs=1) as ppool:
            x_sb = ppool.tile([128, NJ * F], F32, tag="xsb")
            xs16_sb = ppool.tile([128, NJ * F], F16, tag="xs16")
            x_src = x_d.ap().rearrange("(q j) f -> q j f", q=128)
            xs_dst = xs_d.ap().rearrange("(q j) f -> q j f", q=128)[:, :, 0:F]
            for c in range(PCH):
                nc.sync.dma_start(
                    x_sb[:, c * PW * F:(c + 1) * PW * F].rearrange(
                        "p (s f) -> p s f", f=F),
                    x_src[:, c * PW:(c + 1) * PW, :],
                )
                rs_bc = bass.AP(
                    rs_or_sb.tensor, rs_or_sb.offset + c * PW,
                    [rs_or_sb.ap[0], [1, PW], [0, F]])
                nc.vector.tensor_tensor(
                    xs16_sb[:, c * PW * F:(c + 1) * PW * F].rearrange(
                        "p (s f) -> p s f", f=F),
                    x_sb[:, c * PW * F:(c + 1) * PW * F].rearrange(
                        "p (s f) -> p s f", f=F),
                    rs_bc, op=ALU.mult)
                nc.scalar.dma_start(
                    xs_dst[:, c * PW:(c + 1) * PW, :],
                    xs16_sb[:, c * PW * F:(c + 1) * PW * F].rearrange(
                        "p (s f) -> p s f", f=F),
                )

        if USE_BARRIERS:
            tc.strict_bb_all_engine_barrier()

        # ---- tail pools (P, S, BN sums), usable inside layer 2
        lwpool = ctx.enter_context(tc.tile_pool(name="lw", bufs=6))
        pp_pool = ctx.enter_context(
            tc.tile_pool(name="ppsum", bufs=1, space="PSUM"))
        psum1 = pp_pool.tile([F + 1, 512], F32, tag="ps1", name="ps1")
        psum2 = pp_pool.tile([F + 1, CF - 512], F32, tag="ps2", name="ps2")
        psum3 = pp_pool.tile([F + 1, F + 1], F32, tag="ps3", name="ps3")


        # ---- two conv layers
        for layer in range(2):
            src_d = xs_d if layer == 0 else h1_d
            w_sb = w1_sb if layer == 0 else w2_sb
            b_sb = b1_sb if layer == 0 else b2_sb
            pend_a = pend_b = None
            wl_q = []
            wpool0 = [None]
            stpool0 = [None]

            stage_map = {}

            def back_half(s, pt, layer=layer, b_sb=b_sb):
                if layer == 0:
                    t3 = wpool0[0].tile([128, F], F16, tag="t3")
                    nc.vector.scalar_tensor_tensor(
                        t3[:], pt[:], rs_in_sb[:, s:s + 1], b_sb,
                        op0=ALU.mult, op1=ALU.add)
                    g = s // GSL
                    if g not in stage_map:
                        stage_map[g] = stpool0[0].tile(
                            [128, GSL * F], F16, tag="stage", name="stage")
                    stage = stage_map[g]
                    s_loc = s - g * GSL
                    nc.scalar.activation(
                        stage[:, s_loc * F:(s_loc + 1) * F], t3[:],
                        AF.Relu, scale=rs_out_sb[:, s:s + 1])
                    if s_loc == GSL - 1 or s == NS - 1:
                        dst_ap = h1_d.ap().rearrange(
                            "(s p) f -> p s f", p=128)[:, :, 0:F]
                        nc.sync.dma_start(
                            dst_ap[:, g * GSL:g * GSL + s_loc + 1, :],
                            stage[:, 0:(s_loc + 1) * F].rearrange(
                                "p (a f) -> p a f", f=F))
                        del stage_map[g]
                else:
                    hc = stpool0[0].tile([128, 65], F16, tag="hc")
                    nc.gpsimd.memset(hc[:, F:F + 1], 1.0)
                    nc.vector.scalar_tensor_tensor(
                        hc[:, 0:F], pt[:], rs_in_sb[:, s:s + 1],
                        b_sb, op0=ALU.mult, op1=ALU.add)
                    if debug:
                        nc.sync.dma_start(
                            h2_d.ap()[:, s * 65:(s + 1) * 65], hc[:])
                    st = hc[:]
                    wl = wl_q[s]
                    kw = dict(start=(s == 0), stop=(s == NS - 1),
                              skip_group_check=True)
                    nc.tensor.matmul(psum1[:], st, wl[:, 0:512], **kw)
                    nc.tensor.matmul(psum2[:], st, wl[:, 512:CF], **kw)
                    nc.tensor.matmul(psum3[:], st, st, **kw)
            with ExitStack() as lctx:
                gpool = lctx.enter_context(tc.tile_pool(name=f"g{layer}", bufs=3))
                ohpool = lctx.enter_context(tc.tile_pool(name=f"oh{layer}", bufs=3))
                wpool = lctx.enter_context(tc.tile_pool(name=f"wk{layer}", bufs=4))
                stpool = lctx.enter_context(tc.tile_pool(name=f"st{layer}", bufs=4))
                wpool0[0] = wpool
                stpool0[0] = stpool
                pa_pool = lctx.enter_context(
                    tc.tile_pool(name=f"pa{layer}", bufs=2, space="PSUM"))
                pb_pool = lctx.enter_context(
                    tc.tile_pool(name=f"pb{layer}", bufs=2, space="PSUM"))

                for (s0, gsz) in GROUPS:
                    gt = gpool.tile([128, gsz * NBU * 128], F16,
                                    tag=f"gt{gsz}")
                    nc.gpsimd.dma_gather(
                        out_ap=gt[:].rearrange("p (j f) -> p j f", f=128),
                        in_ap=src_d.ap(),
                        idxs_ap=idx_sb[:, s0 * NBU * 8:(s0 + gsz) * NBU * 8],
                        num_idxs=gsz * NBU * 128,
                        num_idxs_reg=gsz * NBU * 128,
                        elem_size=128,
                        single_packet=False,
                    )
                    if npb:
                        gtp = gpool.tile([128, gsz * npb * 256], F16,
                                         tag=f"gtp{gsz}")
                        nc.gpsimd.dma_gather(
                            out_ap=gtp[:].rearrange("p (j f) -> p j f", f=256),
                            in_ap=bass.AP(src_d, 0, [[128, n_nodes - 1],
                                                     [1, 256]]),
                            idxs_ap=idx_sb[:, PB + s0 * npb * 8:
                                           PB + (s0 + gsz) * npb * 8],
                            num_idxs=gsz * npb * 128,
                            num_idxs_reg=gsz * npb * 128,
                            elem_size=256,
                            elem_step=128,
                            single_packet=False,
                        )
                    for s_loc in range(gsz):
                        s = s0 + s_loc
                        if layer != 0 and INTERLEAVE_TAIL:
                            # prefetch lin_W two slices ahead of its use
                            wl = lwpool.tile([128, CF], F16, tag="wl",
                                             name="wl")
                            nc.scalar.dma_start(
                                wl[:], lw_d.ap()[s * 128:(s + 1) * 128, :])
                            wl_q.append(wl)
                        oh = ohpool.tile([128, NBLK * 128], F16, tag="oh")
                        for k in range(NBLK):
                            nc.vector.tensor_scalar(
                                oh[:, k * 128:(k + 1) * 128], iota_sb,
                                dstloc_sb[:, s * NBLK + k:s * NBLK + k + 1],
                                None, op0=ALU.is_equal)
                        # scatter: aggT[f, n] = sum_e gt[e, f] * oh[e, n]
                        pa = pa_pool.tile([F, 128], F32, tag="pa")
                        for k in range(NBU):
                            j = s_loc * NBU + k
                            nc.tensor.matmul(
                                pa[:], gt[:, j * 128:j * 128 + F],
                                oh[:, k * 128:(k + 1) * 128],
                                start=(k == 0),
                                stop=(npb == 0 and k == NBU - 1))
                        for i in range(npb):
                            pb0 = (s_loc * npb + i) * 256
                            kk = NBU + 2 * i
                            nc.tensor.matmul(
                                pa[:], gtp[:, pb0:pb0 + F],
                                oh[:, kk * 128:(kk + 1) * 128],
                                start=False, stop=False)
                            nc.tensor.matmul(
                                pa[:], gtp[:, pb0 + 128:pb0 + 128 + F],
                                oh[:, (kk + 1) * 128:(kk + 2) * 128],
                                start=False, stop=(i == npb - 1))
                        # software pipeline: aggTs copy of s-1 (its scatters
                        # long done -> no DVE stall), conv of s-1 (aggTs just
                        # copied -> no PE stall), epilogue+tail of s-2
                        if pend_a is not None:
                            ps, ppa = pend_a
                            aggTs = wpool.tile([F, 128], F16, tag="aggTs")
                            nc.vector.tensor_copy(aggTs[:], ppa[:])
                            pt = pb_pool.tile([128, F], F32, tag="pt")
                            nc.tensor.matmul(pt[:], aggTs[:], w_sb)
                            if pend_b is not None:
                                back_half(*pend_b)
                            pend_b = (ps, pt)
                        pend_a = (s, pa)
                # drain the two pipeline stages (inside the pool scope)
                if pend_a is not None:
                    ps, ppa = pend_a
                    aggTs = wpool.tile([F, 128], F16, tag="aggTs")
                    nc.vector.tensor_copy(aggTs[:], ppa[:])
                    pt = pb_pool.tile([128, F], F32, tag="pt")
                    nc.tensor.matmul(pt[:], aggTs[:], w_sb)
                    if pend_b is not None:
                        back_half(*pend_b)
                    back_half(ps, pt)
                elif pend_b is not None:
                    back_half(*pend_b)
            if USE_BARRIERS and layer == 0:
                tc.strict_bb_all_engine_barrier()

        # ---- finalize: extract P (diag via mask), S, s1, s2
        with tc.tile_pool(name="fin", bufs=1) as fpool, \
                tc.tile_pool(name="finp", bufs=1, space="PSUM") as fpp:
            mm1 = fpool.tile([F, 512], F16, tag="mm1")
            mm2 = fpool.tile([F, CF - 512], F16, tag="mm2")
            mm3 = fpool.tile([F, F], F16, tag="mm3")
            nc.vector.tensor_tensor(mm1[:], psum1[0:F, :], mask_sb[:, 0:512],
                                    op=ALU.mult)
            nc.vector.tensor_tensor(mm2[:], psum2[0:F, :], mask_sb[:, 512:CF],
                                    op=ALU.mult)
            # mask[:, 0:64] is the identity block (c=0)
            nc.vector.tensor_tensor(mm3[:], psum3[0:F, 0:F], mask_sb[:, 0:F],
                                    op=ALU.mult)
            pP1 = fpp.tile([1, 512], F32, tag="pP1", name="pP1")
            pP2 = fpp.tile([1, CF - 512], F32, tag="pP2", name="pP2")
            pP3 = fpp.tile([1, F], F32, tag="pP3", name="pP3")
            nc.tensor.matmul(pP1[:], ones_sb, mm1[:])
            nc.tensor.matmul(pP2[:], ones_sb, mm2[:])
            nc.tensor.matmul(pP3[:], ones_sb, mm3[:])
            out_sb = fpool.tile([1, 2 * CF + 2 * F], F32, tag="outsb")
            nc.vector.tensor_copy(out_sb[:, 0:512], pP1[:])
            nc.vector.tensor_copy(out_sb[:, 512:CF], pP2[:])
            nc.vector.tensor_copy(out_sb[:, CF:CF + 512], psum1[F:F + 1, :])
            nc.vector.tensor_copy(out_sb[:, CF + 512:2 * CF],
                                  psum2[F:F + 1, :])
            nc.vector.tensor_copy(out_sb[:, 2 * CF:2 * CF + F],
                                  psum3[F:F + 1, 0:F])
            nc.vector.tensor_copy(out_sb[:, 2 * CF + F:2 * CF + 2 * F], pP3[:])
            nc.sync.dma_start(out_d.ap(), out_sb[:])

    nc.compile()
    return nc


_PROGRAM_CACHE = {}


def _get_program(key):
    if key not in _PROGRAM_CACHE:
        _PROGRAM_CACHE[key] = _build_program(*key)
    return _PROGRAM_CACHE[key]


def gcn_forward(x, edge_src, edge_dst, W1, b1, W2, b2, bn_gamma, bn_beta,
                lin_W, lin_b, gsl=None):
    """Full forward pass. x [B, N, F]; returns [B, C]."""
    x = np.asarray(x, np.float32)
    edge_src = np.asarray(edge_src)
    edge_dst = np.asarray(edge_dst)
    W1 = np.asarray(W1, np.float32)
    b1 = np.asarray(b1, np.float32)
    W2 = np.asarray(W2, np.float32)
    b2 = np.asarray(b2, np.float32)
    bn_gamma = np.asarray(bn_gamma, np.float32)
    bn_beta = np.asarray(bn_beta, np.float32)
    lin_W = np.asarray(lin_W, np.float32)
    lin_b = np.asarray(lin_b, np.float32)

    B, N, F = x.shape
    C = lin_W.shape[0]
    NS = N // 128
    n_cores = B
    CF = C * F

    # first pass: relabel all graphs, find shared EPS
    preps = []
    max_cnt = 1
    for b in range(B):
        p = _prep_graph(edge_src[b].astype(np.int64),
                        edge_dst[b].astype(np.int64), N)
        preps.append(p)
        max_cnt = max(max_cnt, p["max_cnt"])
    EPS = ((max_cnt + 127) // 128) * 128
    if gsl is None:
        gsl = 4
        while NS % gsl or gsl * EPS > 9216:
            gsl //= 2
            if gsl == 0:
                gsl = 1
                break

    def pad128(a):
        out = np.zeros((128, a.shape[1]), a.dtype)
        out[:a.shape[0]] = a
        return out

    iota = np.tile(np.arange(128, dtype=np.float16), (128, 1))
    mask = np.zeros((F, CF), np.float16)
    for f in range(F):
        mask[f, f::F] = 1.0
    ones64 = np.ones((F, 1), np.float16)
    b1b = np.tile(b1, (128, 1)).astype(np.float32)
    b2b = np.tile(b2, (128, 1)).astype(np.float32)
    cb16 = np.concatenate([
        iota, pad128(W1.astype(np.float16)), pad128(W2.astype(np.float16)),
        pad128(mask), pad128(ones64)], axis=1)
    lwr = lin_W.reshape(C, N, F)

    npb = min(3, (EPS // 128 - 1) // 2)
    while npb > 0:
        if all(_finish_prep(preps[b], N, EPS, npb=npb) for b in range(B)):
            break
        npb -= 1
    if npb == 0:
        for b in range(B):
            _finish_prep(preps[b], N, EPS, npb=0)

    nc = _get_program((N, F, EPS, C, n_cores, gsl, npb))

    in_maps = []
    for b in range(B):
        p = preps[b]
        inv = p["inv"]
        lw16 = np.ascontiguousarray(
            lwr[:, inv, :].transpose(1, 0, 2).reshape(N, CF)).astype(np.float16)
        cb32 = np.concatenate([
            b1b, b2b, p["rs_out_col"], p["rs_in_col"], p["rs_or"],
            p["dstloc"]], axis=1).astype(np.float32)
        in_maps.append({
            "x": np.ascontiguousarray(x[b][inv]),
            "idx": p["idx16"],
            "cb32": cb32,
            "cb16": cb16,
            "lw16": lw16,
        })

    res = run_bass_kernel_spmd(nc, in_maps, core_ids=list(range(n_cores)))

    P = np.zeros((B, C, F), np.float64)
    s1 = np.zeros(F, np.float64)
    s2 = np.zeros(F, np.float64)
    S = None
    for b in range(B):
        o = res.results[b]["out"][0].astype(np.float64)
        P[b] = o[:CF].reshape(C, F)
        s1 += o[2 * CF:2 * CF + F]
        s2 += o[2 * CF + F:2 * CF + 2 * F]
        if S is None:
            S = o[CF:2 * CF].reshape(C, F)

    cnt = B * N
    mean = s1 / cnt
    var = s2 / cnt - mean * mean
    a = bn_gamma / np.sqrt(var + BN_EPS)
    d = bn_beta - mean * a
    out = (P * a[None, None, :]).sum(-1) + (S * d[None, :]).sum(-1)[None, :] \
        + lin_b[None, :]
    return out.astype(np.float32)


def kernel(**inputs):
    return gcn_forward(
        inputs["x"], inputs["edge_src"], inputs["edge_dst"],
        inputs["W1"], inputs["b1"], inputs["W2"], inputs["b2"],
        inputs["bn_gamma"], inputs["bn_beta"], inputs["lin_W"], inputs["lin_b"])
